# revision 27
# baseline (speedup 1.0000x reference)
"""Dual-pixel depth-merge (forward splat) kernel for Trainium2, 8 NeuronCores.

Math: for integer pixel grid x, the reference computes pos = fl(x +- depth)
(f32-rounded), x0 = floor(pos), f = pos - x0. Define the per-view fractional
offsets
    v_l[i] = fl(i + depth[i]) - i   (exact f32 subtraction, in [0, 8])
    v_r[i] = i - fl(i - depth[i])   (exact f32 subtraction, in [0, 8])
Then each view's splat is a 9-tap shifted weighted sum with hat weights
    Wl_d = relu(1 - |v_l - d|),  Wr_d = relu(1 - |v_r - d|),  d = 0..8:
    count_l[j] = sum_d Wl_d[j-d]      acc_l[c,j] = sum_d (Wl_d*img_c)[j-d]
    count_r[j] = sum_d Wr_d[j+d]      acc_r[c,j] = sum_d (Wr_d*img_c)[j+d]
    left = acc_l / max(count_l, eps)  right = acc_r / max(count_r, eps)
This reproduces the reference's weights bit-for-bit (matching its f32
rounding of x+-depth), so count==0 happens exactly where the reference's
does — and there acc==0 too, making the eps-divide equal the reference's
where(count==0, 1, count).

Sharding: pure data parallel over h (the scatter is along w only) — core m
takes h rows [m*128, (m+1)*128) for all batches. No halo, no communication.
"""

import numpy as np

import concourse.bacc as bacc
import concourse.bass as bass
import concourse.mybir as mybir
import concourse.tile as tile
from concourse.bass_utils import run_bass_kernel_spmd

B, C, H, W = 4, 3, 1024, 1024
NCORES = 8
HS = H // NCORES  # 128 h-rows per core
NTAP = 9
F32 = mybir.dt.float32
EPS = 1e-20

_MAX = mybir.AluOpType.max
_ADD = mybir.AluOpType.add
_SUB = mybir.AluOpType.subtract
_RELU = mybir.ActivationFunctionType.Relu
_ABS = mybir.ActivationFunctionType.Abs


def _bcast_c(ap):
    """View a [HS, W] tile as [HS, C, W] by repeating along a step-0 dim."""
    a = ap.ap
    return bass.AP(tensor=ap.tensor, offset=ap.offset, ap=[list(a[0]), [0, C], list(a[1])])


CFG = {
    "io": 2, "w": 4, "v": 2, "t": 2, "p": 3, "acc": 2, "accn": 2,
    # Column split: DVE handles w in [0, spl), GPSIMD handles [spl, W), for
    # products/adds (spl_p) and count sums (spl_c).
    "spl_p": 672,
    "spl_c": 704,
}


def build_program(skip: frozenset = frozenset(), cfg: dict | None = None) -> bass.Bass:
    """skip: debug knob — subset of {"counts", "prods", "weights"} to omit
    (produces wrong results; used only for critical-path bisection)."""
    cfg = {**CFG, **(cfg or {})}
    nc = bacc.Bacc()
    image = nc.dram_tensor("image", [B, C, HS, W], F32, kind="ExternalInput")
    depth = nc.dram_tensor("depth", [B, HS, W], F32, kind="ExternalInput")
    left = nc.dram_tensor("left", [B, C, HS, W], F32, kind="ExternalOutput")
    right = nc.dram_tensor("right", [B, C, HS, W], F32, kind="ExternalOutput")

    with tile.TileContext(nc) as tc:
        with (
            tc.tile_pool(name="consts", bufs=1) as c_pool,
            tc.tile_pool(name="io", bufs=cfg["io"]) as io_pool,
            tc.tile_pool(name="wts", bufs=cfg["w"]) as w_pool,
            tc.tile_pool(name="voff", bufs=cfg["v"]) as v_pool,
            tc.tile_pool(name="tmp", bufs=cfg["t"]) as t_pool,
            tc.tile_pool(name="prod", bufs=cfg["p"]) as p_pool,
            tc.tile_pool(name="accs", bufs=cfg["acc"]) as acc_pool,
            tc.tile_pool(name="accn", bufs=cfg["accn"]) as accn_pool,
        ):
            # Per-tap bias constants and the column-index (iota) row.
            negd = c_pool.tile([HS, NTAP], F32, tag="negd")
            for d in range(NTAP):
                nc.vector.memset(negd[:, d : d + 1], -float(d))
            iota_i = t_pool.tile([HS, W], mybir.dt.int32, tag="t")
            nc.gpsimd.iota(iota_i[:], [[1, W]], channel_multiplier=0)
            iota = c_pool.tile([HS, W], F32, tag="iota")
            nc.vector.tensor_copy(iota[:], iota_i[:])

            for b in range(B):
                dep = io_pool.tile([HS, W], F32, tag="dep")
                nc.sync.dma_start(out=dep[:], in_=depth[b])
                img = io_pool.tile([HS, C, W], F32, tag="img")
                nc.sync.dma_start(out=img[:], in_=image[b].transpose([1, 0, 2]))

                # Exact per-view fractional offsets (reproduce reference's
                # f32 rounding of x +- depth; the second subtract is exact).
                vl = v_pool.tile([HS, W], F32, tag="vl")
                vr = v_pool.tile([HS, W], F32, tag="vr")
                s = t_pool.tile([HS, W], F32, tag="s")
                nc.vector.tensor_tensor(s[:], dep[:], iota[:], _ADD)
                nc.vector.tensor_tensor(vl[:], s[:], iota[:], _SUB)
                s2 = t_pool.tile([HS, W], F32, tag="s")
                nc.vector.tensor_tensor(s2[:], iota[:], dep[:], _SUB)
                nc.vector.tensor_tensor(vr[:], iota[:], s2[:], _SUB)

                # Interleave the two views tap-by-tap so DVE/GPSIMD/ACT all
                # stay fed. Work is column-split: DVE takes [0, spl),
                # GPSIMD [spl, W) of every product/add/count op.
                sp = cfg["spl_p"]
                sc = cfg["spl_c"]
                views = (("l", vl), ("r", vr))
                cnt_l = accn_pool.tile([HS, W], F32, tag="cl")
                cnt_r = accn_pool.tile([HS, W], F32, tag="cr")
                acc_l = acc_pool.tile([HS, C, W], F32, tag="al")
                acc_r = acc_pool.tile([HS, C, W], F32, tag="ar")
                cnts = {"l": cnt_l, "r": cnt_r}
                accs = {"l": acc_l, "r": acc_r}
                for d in range(NTAP):
                    for view, v in views:
                        cnt, acc = cnts[view], accs[view]
                        # Tap weight W_d = relu(1 - |v - d|) on the scalar engine.
                        td = t_pool.tile([HS, W], F32, tag="t")
                        nc.scalar.activation(td[:], v[:], _ABS, bias=negd[:, d : d + 1], scale=1.0)
                        wd = w_pool.tile([HS, W], F32, tag="w")
                        nc.scalar.activation(wd[:], td[:], _RELU, bias=1.0, scale=-1.0)
                        if d == 0:
                            nc.vector.tensor_scalar(cnt[:, 0:sc], wd[:, 0:sc], 0.0, None, _ADD)
                            nc.gpsimd.tensor_scalar(cnt[:, sc:W], wd[:, sc:W], 0.0, None, _ADD)
                            nc.vector.tensor_mul(acc[:, :, 0:sp], _bcast_c(wd[:, 0:sp]), img[:, :, 0:sp])
                            nc.gpsimd.tensor_mul(acc[:, :, sp:W], _bcast_c(wd[:, sp:W]), img[:, :, sp:W])
                            continue
                        if "counts" not in skip:
                            if view == "l":
                                nc.vector.tensor_tensor(cnt[:, d:sc], cnt[:, d:sc], wd[:, 0 : sc - d], _ADD)
                                nc.gpsimd.tensor_tensor(cnt[:, sc:W], cnt[:, sc:W], wd[:, sc - d : W - d], _ADD)
                            else:
                                nc.vector.tensor_tensor(cnt[:, 0:sc], cnt[:, 0:sc], wd[:, d : sc + d], _ADD)
                                nc.gpsimd.tensor_tensor(cnt[:, sc : W - d], cnt[:, sc : W - d], wd[:, sc + d : W], _ADD)
                        if "prods" not in skip:
                            pd = p_pool.tile([HS, C, W], F32, tag="p")
                            nc.vector.tensor_mul(pd[:, :, 0:sp], _bcast_c(wd[:, 0:sp]), img[:, :, 0:sp])
                            nc.gpsimd.tensor_mul(pd[:, :, sp:W], _bcast_c(wd[:, sp:W]), img[:, :, sp:W])
                            if view == "l":
                                nc.vector.tensor_add(acc[:, :, d:sp], acc[:, :, d:sp], pd[:, :, 0 : sp - d])
                                nc.gpsimd.tensor_add(acc[:, :, sp:W], acc[:, :, sp:W], pd[:, :, sp - d : W - d])
                            else:
                                nc.vector.tensor_add(acc[:, :, 0:sp], acc[:, :, 0:sp], pd[:, :, d : sp + d])
                                nc.gpsimd.tensor_add(acc[:, :, sp : W - d], acc[:, :, sp : W - d], pd[:, :, sp + d : W])

                # Normalize: out = acc * (1 / max(count, eps)).
                for view, _ in views:
                    cnt, acc = cnts[view], accs[view]
                    rc = accn_pool.tile([HS, W], F32, tag=f"rc{view}")
                    nc.gpsimd.tensor_scalar(cnt[:], cnt[:], EPS, None, _MAX)
                    nc.vector.reciprocal_approx_fast(out=rc[:], in_=cnt[:])
                    nc.vector.tensor_mul(acc[:], acc[:], _bcast_c(rc))

                nc.sync.dma_start(out=left[b].transpose([1, 0, 2]), in_=accs["l"][:])
                nc.sync.dma_start(out=right[b].transpose([1, 0, 2]), in_=accs["r"][:])
    nc.compile()
    return nc


_NC_CACHE = None


def _get_program():
    global _NC_CACHE
    if _NC_CACHE is None:
        _NC_CACHE = build_program()
    return _NC_CACHE


def kernel(image: np.ndarray, depth: np.ndarray):
    image = np.ascontiguousarray(image, dtype=np.float32)
    depth = np.ascontiguousarray(depth, dtype=np.float32)
    assert image.shape == (B, C, H, W) and depth.shape == (B, H, W)

    nc = _get_program()
    in_maps = []
    for m in range(NCORES):
        sl = slice(m * HS, (m + 1) * HS)
        in_maps.append(
            {
                "image": np.ascontiguousarray(image[:, :, sl, :]),
                "depth": np.ascontiguousarray(depth[:, sl, :]),
            }
        )
    res = run_bass_kernel_spmd(nc, in_maps, core_ids=list(range(NCORES)))
    left = np.concatenate([r["left"] for r in res.results], axis=2)
    right = np.concatenate([r["right"] for r in res.results], axis=2)
    return left, right


# revision 33
# speedup vs baseline: 1.0281x; 1.0281x over previous
"""Dual-pixel depth-merge (forward splat) kernel for Trainium2, 8 NeuronCores.

Math: for integer pixel grid x, the reference computes pos = fl(x +- depth)
(f32-rounded), x0 = floor(pos), f = pos - x0. Define the per-view fractional
offsets
    v_l[i] = fl(i + depth[i]) - i   (exact f32 subtraction, in [0, 8])
    v_r[i] = i - fl(i - depth[i])   (exact f32 subtraction, in [0, 8])
Then each view's splat is a 9-tap shifted weighted sum with hat weights
    Wl_d = relu(1 - |v_l - d|),  Wr_d = relu(1 - |v_r - d|),  d = 0..8:
    count_l[j] = sum_d Wl_d[j-d]      acc_l[c,j] = sum_d (Wl_d*img_c)[j-d]
    count_r[j] = sum_d Wr_d[j+d]      acc_r[c,j] = sum_d (Wr_d*img_c)[j+d]
    left = acc_l / max(count_l, eps)  right = acc_r / max(count_r, eps)
This reproduces the reference's weights bit-for-bit (matching its f32
rounding of x+-depth), so count==0 happens exactly where the reference's
does — and there acc==0 too, making the eps-divide equal the reference's
where(count==0, 1, count).

Sharding: pure data parallel over h (the scatter is along w only) — core m
takes h rows [m*128, (m+1)*128) for all batches. No halo, no communication.
"""

import numpy as np

import concourse.bacc as bacc
import concourse.bass as bass
import concourse.mybir as mybir
import concourse.tile as tile
from concourse.bass_utils import run_bass_kernel_spmd

B, C, H, W = 4, 3, 1024, 1024
NCORES = 8
HS = H // NCORES  # 128 h-rows per core
NTAP = 9
F32 = mybir.dt.float32
EPS = 1e-20

_MAX = mybir.AluOpType.max
_ADD = mybir.AluOpType.add
_SUB = mybir.AluOpType.subtract
_RELU = mybir.ActivationFunctionType.Relu
_ABS = mybir.ActivationFunctionType.Abs


def _bcast_c(ap):
    """View a [HS, W] tile as [HS, C, W] by repeating along a step-0 dim."""
    a = ap.ap
    return bass.AP(tensor=ap.tensor, offset=ap.offset, ap=[list(a[0]), [0, C], list(a[1])])


CFG = {
    "io": 2, "w": 4, "v": 2, "t": 2, "p": 3, "acc": 2, "accn": 2,
    # Column split: DVE handles w in [0, spl), GPSIMD handles [spl, W), for
    # products/adds (spl_p) and count sums (spl_c).
    "spl_p": 688,
    "spl_a": 672,
    "spl_c": 704,
}


def build_program(skip: frozenset = frozenset(), cfg: dict | None = None) -> bass.Bass:
    """skip: debug knob — subset of {"counts", "prods", "weights"} to omit
    (produces wrong results; used only for critical-path bisection)."""
    cfg = {**CFG, **(cfg or {})}
    nc = bacc.Bacc()
    image = nc.dram_tensor("image", [B, C, HS, W], F32, kind="ExternalInput")
    depth = nc.dram_tensor("depth", [B, HS, W], F32, kind="ExternalInput")
    left = nc.dram_tensor("left", [B, C, HS, W], F32, kind="ExternalOutput")
    right = nc.dram_tensor("right", [B, C, HS, W], F32, kind="ExternalOutput")

    with tile.TileContext(nc) as tc:
        with (
            tc.tile_pool(name="consts", bufs=1) as c_pool,
            tc.tile_pool(name="io", bufs=cfg["io"]) as io_pool,
            tc.tile_pool(name="wts", bufs=cfg["w"]) as w_pool,
            tc.tile_pool(name="voff", bufs=cfg["v"]) as v_pool,
            tc.tile_pool(name="tmp", bufs=cfg["t"]) as t_pool,
            tc.tile_pool(name="prod", bufs=cfg["p"]) as p_pool,
            tc.tile_pool(name="accs", bufs=cfg["acc"]) as acc_pool,
            tc.tile_pool(name="accn", bufs=cfg["accn"]) as accn_pool,
        ):
            # Per-tap bias constants and the column-index (iota) row.
            negd = c_pool.tile([HS, NTAP], F32, tag="negd")
            for d in range(NTAP):
                nc.vector.memset(negd[:, d : d + 1], -float(d))
            iota_i = t_pool.tile([HS, W], mybir.dt.int32, tag="t")
            nc.gpsimd.iota(iota_i[:], [[1, W]], channel_multiplier=0)
            iota = c_pool.tile([HS, W], F32, tag="iota")
            nc.vector.tensor_copy(iota[:], iota_i[:])

            for b in range(B):
                dep = io_pool.tile([HS, W], F32, tag="dep")
                nc.sync.dma_start(out=dep[:], in_=depth[b])
                img = io_pool.tile([HS, C, W], F32, tag="img")
                nc.sync.dma_start(out=img[:], in_=image[b].transpose([1, 0, 2]))

                # Exact per-view fractional offsets (reproduce reference's
                # f32 rounding of x +- depth; the second subtract is exact).
                vl = v_pool.tile([HS, W], F32, tag="vl")
                vr = v_pool.tile([HS, W], F32, tag="vr")
                s = t_pool.tile([HS, W], F32, tag="s")
                nc.vector.tensor_tensor(s[:], dep[:], iota[:], _ADD)
                nc.vector.tensor_tensor(vl[:], s[:], iota[:], _SUB)
                s2 = t_pool.tile([HS, W], F32, tag="s")
                nc.gpsimd.tensor_tensor(s2[:], iota[:], dep[:], _SUB)
                nc.gpsimd.tensor_tensor(vr[:], iota[:], s2[:], _SUB)

                # Interleave the two views tap-by-tap so DVE/GPSIMD/ACT all
                # stay fed. Work is column-split: DVE takes [0, spl),
                # GPSIMD [spl, W) of every product/add/count op.
                sp = cfg["spl_p"]
                sa = cfg["spl_a"]
                sc = cfg["spl_c"]
                views = (("l", vl), ("r", vr))
                cnt_l = accn_pool.tile([HS, W], F32, tag="cl")
                cnt_r = accn_pool.tile([HS, W], F32, tag="cr")
                acc_l = acc_pool.tile([HS, C, W], F32, tag="al")
                acc_r = acc_pool.tile([HS, C, W], F32, tag="ar")
                cnts = {"l": cnt_l, "r": cnt_r}
                accs = {"l": acc_l, "r": acc_r}
                for d in range(NTAP):
                    for view, v in views:
                        cnt, acc = cnts[view], accs[view]
                        # Tap weight W_d = relu(1 - |v - d|) on the scalar engine.
                        td = t_pool.tile([HS, W], F32, tag="t")
                        nc.scalar.activation(td[:], v[:], _ABS, bias=negd[:, d : d + 1], scale=1.0)
                        wd = w_pool.tile([HS, W], F32, tag="w")
                        nc.scalar.activation(wd[:], td[:], _RELU, bias=1.0, scale=-1.0)
                        if d == 0:
                            nc.vector.tensor_scalar(cnt[:, 0:sc], wd[:, 0:sc], 0.0, None, _ADD)
                            nc.gpsimd.tensor_scalar(cnt[:, sc:W], wd[:, sc:W], 0.0, None, _ADD)
                            nc.vector.tensor_mul(acc[:, :, 0:sp], _bcast_c(wd[:, 0:sp]), img[:, :, 0:sp])
                            nc.gpsimd.tensor_mul(acc[:, :, sp:W], _bcast_c(wd[:, sp:W]), img[:, :, sp:W])
                            continue
                        if "counts" not in skip:
                            if view == "l":
                                nc.vector.tensor_tensor(cnt[:, d:sc], cnt[:, d:sc], wd[:, 0 : sc - d], _ADD)
                                nc.gpsimd.tensor_tensor(cnt[:, sc:W], cnt[:, sc:W], wd[:, sc - d : W - d], _ADD)
                            else:
                                nc.vector.tensor_tensor(cnt[:, 0:sc], cnt[:, 0:sc], wd[:, d : sc + d], _ADD)
                                nc.gpsimd.tensor_tensor(cnt[:, sc : W - d], cnt[:, sc : W - d], wd[:, sc + d : W], _ADD)
                        if "prods" not in skip:
                            pd = p_pool.tile([HS, C, W], F32, tag="p")
                            nc.vector.tensor_mul(pd[:, :, 0:sp], _bcast_c(wd[:, 0:sp]), img[:, :, 0:sp])
                            nc.gpsimd.tensor_mul(pd[:, :, sp:W], _bcast_c(wd[:, sp:W]), img[:, :, sp:W])
                            if view == "l":
                                nc.vector.tensor_add(acc[:, :, d:sa], acc[:, :, d:sa], pd[:, :, 0 : sa - d])
                                nc.gpsimd.tensor_add(acc[:, :, sa:W], acc[:, :, sa:W], pd[:, :, sa - d : W - d])
                            else:
                                nc.vector.tensor_add(acc[:, :, 0:sa], acc[:, :, 0:sa], pd[:, :, d : sa + d])
                                nc.gpsimd.tensor_add(acc[:, :, sa : W - d], acc[:, :, sa : W - d], pd[:, :, sa + d : W])

                # Normalize: out = acc * (1 / max(count, eps)).
                for view, _ in views:
                    cnt, acc = cnts[view], accs[view]
                    rc = accn_pool.tile([HS, W], F32, tag=f"rc{view}")
                    nc.gpsimd.tensor_scalar(cnt[:], cnt[:], EPS, None, _MAX)
                    nc.vector.reciprocal_approx_fast(out=rc[:], in_=cnt[:])
                    nc.vector.tensor_mul(acc[:, :, 0:sa], acc[:, :, 0:sa], _bcast_c(rc[:, 0:sa]))
                    nc.gpsimd.tensor_mul(acc[:, :, sa:W], acc[:, :, sa:W], _bcast_c(rc[:, sa:W]))

                nc.sync.dma_start(out=left[b].transpose([1, 0, 2]), in_=accs["l"][:])
                nc.sync.dma_start(out=right[b].transpose([1, 0, 2]), in_=accs["r"][:])
    nc.compile()
    return nc


_NC_CACHE = None


def _get_program():
    global _NC_CACHE
    if _NC_CACHE is None:
        _NC_CACHE = build_program()
    return _NC_CACHE


def kernel(image: np.ndarray, depth: np.ndarray):
    image = np.ascontiguousarray(image, dtype=np.float32)
    depth = np.ascontiguousarray(depth, dtype=np.float32)
    assert image.shape == (B, C, H, W) and depth.shape == (B, H, W)

    nc = _get_program()
    in_maps = []
    for m in range(NCORES):
        sl = slice(m * HS, (m + 1) * HS)
        in_maps.append(
            {
                "image": np.ascontiguousarray(image[:, :, sl, :]),
                "depth": np.ascontiguousarray(depth[:, sl, :]),
            }
        )
    res = run_bass_kernel_spmd(nc, in_maps, core_ids=list(range(NCORES)))
    left = np.concatenate([r["left"] for r in res.results], axis=2)
    right = np.concatenate([r["right"] for r in res.results], axis=2)
    return left, right
